# revision 1
# baseline (speedup 1.0000x reference)
"""Trainium2 Bass kernel for nn_NDNRefinement (4-layer GNN message passing).

Strategy (8 NeuronCores):
- Sort triples by s_idx on host; shard triples so core c owns triples whose
  subject falls in its object range [c*OS, (c+1)*OS). Subject-side pooling is
  then core-local. Object-side rows are exchanged via a fixed-size-bucket
  AllToAll. Per-object MLPs are data-parallel over the object shard; the new
  object table is AllGathered between layers.
- Activations are kept feature-major (features on partitions) through the MLP;
  W1b outputs are produced entry-major so pooling can be done with one-hot
  matmuls (iota + is_equal trick). All matmuls run in bf16 with fp32 PSUM.
- All loop structure is static and identical across cores (SPMD); per-core
  raggedness is absorbed by host-computed padding (padded entries have
  one-hot local id -1 so they contribute zero to pooling).
"""

import os
import numpy as np
import ml_dtypes

import concourse.bass as bass
import concourse.bacc as bacc
import concourse.tile as tile
from concourse import mybir
from concourse.bass_utils import run_bass_kernel_spmd
from concourse.masks import make_identity

BF16 = mybir.dt.bfloat16
F32 = mybir.dt.float32
I32 = mybir.dt.int32
P = 128
NC = 8
ALPHA = 0.2

# (din, h, dout) per layer
DIMS = [(64, 512, 128), (128, 512, 128), (128, 512, 128), (128, 128, 128)]


def _rup(x, m):
    return ((int(x) + m - 1) // m) * m


# ---------------------------------------------------------------------------
# Host preprocessing
# ---------------------------------------------------------------------------

def preprocess(inputs):
    """Compute the static schedule + per-core input maps from full inputs."""
    obj_vecs = np.asarray(inputs["obj_vecs"], np.float32)
    pred_vecs = np.asarray(inputs["pred_vecs"], np.float32)
    pred_boxes = np.asarray(inputs["pred_boxes"], np.float32)
    s_idx = np.asarray(inputs["s_idx"], np.int32)
    o_idx = np.asarray(inputs["o_idx"], np.int32)

    O = obj_vecs.shape[0]
    assert O % NC == 0
    OS = O // NC                      # real objects per core
    OSP = _rup(OS, P)                 # padded objects per core
    NT = OSP // P                     # object tiles per core
    OG = NC * OSP                     # padded global object count

    def gmap(idx):
        return ((idx // OS) * OSP + (idx % OS)).astype(np.int32)

    order = np.argsort(s_idx, kind="stable")
    s_sorted = s_idx[order]
    o_sorted = o_idx[order]
    bnd = np.searchsorted(s_sorted, np.arange(NC + 1) * OS)
    counts_c = np.diff(bnd)
    T_PC = max(_rup(counts_c.max(), 512), 512)
    NB = T_PC // 512

    # per-object counts (same every layer)
    cnt = np.bincount(s_idx, minlength=O) + np.bincount(o_idx, minlength=O)
    inv_cnt = (1.0 / np.maximum(cnt, 1)).astype(np.float32)

    percore = []
    maxB = 0
    max_s_load = 0
    for c in range(NC):
        sl = slice(bnd[c], bnd[c + 1])
        idxs = order[sl]
        n = len(idxs)
        s_c = s_sorted[sl]
        o_c = o_sorted[sl]
        d_c = (o_c // OS).astype(np.int64)

        # bucket ranks: entries sorted by (dest, o)
        ordb = np.lexsort((o_c, d_c))
        rank = np.empty(n, np.int64)
        d_srt = d_c[ordb]
        first = np.searchsorted(d_srt, np.arange(NC))
        rank[ordb] = np.arange(n) - first[d_srt]
        bc = np.bincount(d_c, minlength=NC)
        maxB = max(maxB, int(bc.max()) if n else 0)

        sload = np.bincount((s_c - c * OS) // P, minlength=NT)
        max_s_load = max(max_s_load, int(sload.max()) if n else 0)
        percore.append(dict(idxs=idxs, n=n, s_c=s_c, o_c=o_c, d_c=d_c,
                            rank=rank, bc=bc))

    # each bucket keeps headroom at the top for pad-entry trash writes
    n_pad_max = max(int(T_PC - pc["n"]) for pc in percore)
    S_B = _rup(maxB + n_pad_max // NC + 2, P)

    # o-side tile loads need recv layout; compute per dest core
    o_tiles_per_core = []
    max_o_load = 0
    for c in range(NC):
        rows_all, locs_all = [], []
        for d in range(NC):
            pc = percore[d]
            m = pc["d_c"] == c
            rows_all.append(d * S_B + pc["rank"][m])
            locs_all.append(pc["o_c"][m] - c * OS)
        rows_all = np.concatenate(rows_all)
        locs_all = np.concatenate(locs_all)
        oload = np.bincount(locs_all // P, minlength=NT)
        max_o_load = max(max_o_load, int(oload.max()) if len(locs_all) else 0)
        o_tiles_per_core.append((rows_all, locs_all))

    PS = max(1, -(-int(max_s_load) // P))
    PO = max(1, -(-int(max_o_load) // P))

    cfg = dict(O=O, OS=OS, OSP=OSP, NT=NT, OG=OG, T_PC=T_PC, NB=NB,
               S_B=S_B, PS=PS, PO=PO)

    # ---- weights, shared across cores ----
    bf = ml_dtypes.bfloat16
    shared = {}
    shared["w_emb"] = np.asarray(inputs["W_emb"], np.float32).astype(bf)
    shared["b_emb"] = np.asarray(inputs["b_emb"], np.float32).reshape(-1, 1)
    for li, (din, h, dout) in enumerate(DIMS):
        b1b = np.asarray(inputs[f"b1b{li}"], np.float32)
        shared[f"w1a{li}"] = np.asarray(inputs[f"W1a{li}"], np.float32).astype(bf)
        shared[f"w1b{li}"] = np.asarray(inputs[f"W1b{li}"], np.float32).astype(bf)
        shared[f"w2a{li}"] = np.asarray(inputs[f"W2a{li}"], np.float32).astype(bf)
        shared[f"w2b{li}"] = np.asarray(inputs[f"W2b{li}"], np.float32).astype(bf)
        shared[f"b1a{li}"] = np.asarray(inputs[f"b1a{li}"], np.float32).reshape(-1, P).T.copy()
        shared[f"b1bp{li}"] = b1b[h:h + dout].reshape(-1, 1).copy()
        shared[f"b1bs{li}"] = np.broadcast_to(b1b[:h].astype(bf), (P, h)).copy()
        shared[f"b1bo{li}"] = np.broadcast_to(b1b[h + dout:].astype(bf), (P, h)).copy()
        shared[f"b2a{li}"] = np.asarray(inputs[f"b2a{li}"], np.float32).reshape(-1, P).T.copy()
        shared[f"b2b{li}"] = np.asarray(inputs[f"b2b{li}"], np.float32).reshape(-1, 1).copy()
    shared["wbb"] = np.asarray(inputs["W_bb"], np.float32).astype(bf)
    shared["bbb"] = np.asarray(inputs["b_bb"], np.float32).reshape(-1, 1)

    # ---- per-core arrays ----
    x_full = np.concatenate([obj_vecs, pred_boxes], axis=1)       # (O, 68)

    def colpack(vals, width):
        """(<=width*128,) -> (128, width); entry e=(chunk, part) at [part, chunk]."""
        a = np.zeros((width * P,), np.int32)
        a[:len(vals)] = vals
        return a.reshape(width, P).T.copy()

    in_maps = []
    for c in range(NC):
        pc = percore[c]
        idxs, n = pc["idxs"], pc["n"]
        m = {}
        xT = np.zeros((68, OSP), bf)
        xT[:, :OS] = x_full[c * OS:(c + 1) * OS].T.astype(bf)
        m["xt"] = xT
        pT = np.zeros((64, T_PC), bf)
        pT[:, :n] = pred_vecs[idxs].T.astype(bf)
        m["pred0"] = pT
        sg_ = np.zeros((T_PC,), np.int32)
        sg_[:n] = gmap(pc["s_c"])
        og_ = np.zeros((T_PC,), np.int32)
        og_[:n] = gmap(pc["o_c"])
        m["sg"] = colpack(sg_, NB * 4)
        m["og"] = colpack(og_, NB * 4)
        # bucket scatter positions; pad entries spread over per-bucket headroom
        ob_ = np.empty((T_PC,), np.int32)
        ob_[:n] = (pc["d_c"] * S_B + pc["rank"]).astype(np.int32)
        npad = T_PC - n
        if npad:
            i = np.arange(npad)
            d = i % NC
            slot = S_B - 1 - (i // NC)
            assert (slot >= pc["bc"][d]).all(), "trash slots collide with data"
            ob_[n:] = (d * S_B + slot).astype(np.int32)
        m["ob"] = colpack(ob_, NB * 4)
        # s-pool schedule: interleaved (id, loc) columns per chunk
        s_loc = pc["s_c"] - c * OS
        sp = np.zeros((NT, PS, 2, P), np.int32)
        sp[:, :, 1, :] = -1
        tstart = np.searchsorted(s_loc, np.arange(NT + 1) * P)
        for t in range(NT):
            a, b = int(tstart[t]), int(tstart[t + 1])
            k = b - a
            ids = np.arange(a, b, dtype=np.int32)
            lcs = (s_loc[a:b] - t * P).astype(np.int32)
            flat_i = np.zeros((PS * P,), np.int32)
            flat_l = np.full((PS * P,), -1, np.int32)
            flat_i[:k] = ids
            flat_l[:k] = lcs
            sp[t, :, 0, :] = flat_i.reshape(PS, P)
            sp[t, :, 1, :] = flat_l.reshape(PS, P)
        m["spil"] = sp.reshape(NT * PS * 2, P).T.copy()
        # o-pool schedule
        rows_all, locs_all = o_tiles_per_core[c]
        op = np.zeros((NT, PO, 2, P), np.int32)
        op[:, :, 1, :] = -1
        tsel = locs_all // P
        for t in range(NT):
            mask = tsel == t
            k = int(mask.sum())
            flat_i = np.zeros((PO * P,), np.int32)
            flat_l = np.full((PO * P,), -1, np.int32)
            flat_i[:k] = rows_all[mask]
            flat_l[:k] = (locs_all[mask] - t * P)
            op[t, :, 0, :] = flat_i.reshape(PO, P)
            op[t, :, 1, :] = flat_l.reshape(PO, P)
        m["opil"] = op.reshape(NT * PO * 2, P).T.copy()
        iv = np.zeros((OSP,), np.float32)
        iv[:OS] = inv_cnt[c * OS:(c + 1) * OS]
        m["invc"] = iv.reshape(NT, P).T.copy()
        m.update(shared)
        in_maps.append(m)

    return cfg, in_maps


# ---------------------------------------------------------------------------
# Kernel builder
# ---------------------------------------------------------------------------

def build_kernel(cfg):
    OSP, NT, OG = cfg["OSP"], cfg["NT"], cfg["OG"]
    T_PC, NB, S_B = cfg["T_PC"], cfg["NB"], cfg["S_B"]
    PS, PO = cfg["PS"], cfg["PO"]

    nc = bacc.Bacc("TRN2", target_bir_lowering=False, debug=False,
                   num_devices=NC)

    # ---- parameters ----
    xt = nc.declare_dram_parameter("xt", [68, OSP], BF16, isOutput=False)
    pred0 = nc.declare_dram_parameter("pred0", [64, T_PC], BF16, isOutput=False)
    sg = nc.declare_dram_parameter("sg", [P, NB * 4], I32, isOutput=False)
    og = nc.declare_dram_parameter("og", [P, NB * 4], I32, isOutput=False)
    ob = nc.declare_dram_parameter("ob", [P, NB * 4], I32, isOutput=False)
    spil = nc.declare_dram_parameter("spil", [P, NT * PS * 2], I32, isOutput=False)
    opil = nc.declare_dram_parameter("opil", [P, NT * PO * 2], I32, isOutput=False)
    invc = nc.declare_dram_parameter("invc", [P, NT], F32, isOutput=False)

    w_emb = nc.declare_dram_parameter("w_emb", [68, 64], BF16, isOutput=False)
    b_emb = nc.declare_dram_parameter("b_emb", [64, 1], F32, isOutput=False)
    wp = {}
    for li, (din, h, dout) in enumerate(DIMS):
        wp[f"w1a{li}"] = nc.declare_dram_parameter(f"w1a{li}", [3 * din, h], BF16, isOutput=False)
        wp[f"w1b{li}"] = nc.declare_dram_parameter(f"w1b{li}", [h, 2 * h + dout], BF16, isOutput=False)
        wp[f"w2a{li}"] = nc.declare_dram_parameter(f"w2a{li}", [h, h], BF16, isOutput=False)
        wp[f"w2b{li}"] = nc.declare_dram_parameter(f"w2b{li}", [h, dout], BF16, isOutput=False)
        wp[f"b1a{li}"] = nc.declare_dram_parameter(f"b1a{li}", [P, h // P], F32, isOutput=False)
        wp[f"b1bp{li}"] = nc.declare_dram_parameter(f"b1bp{li}", [dout, 1], F32, isOutput=False)
        wp[f"b1bs{li}"] = nc.declare_dram_parameter(f"b1bs{li}", [P, h], BF16, isOutput=False)
        wp[f"b1bo{li}"] = nc.declare_dram_parameter(f"b1bo{li}", [P, h], BF16, isOutput=False)
        wp[f"b2a{li}"] = nc.declare_dram_parameter(f"b2a{li}", [P, h // P], F32, isOutput=False)
        wp[f"b2b{li}"] = nc.declare_dram_parameter(f"b2b{li}", [dout, 1], F32, isOutput=False)
    wbb = nc.declare_dram_parameter("wbb", [P, 4], BF16, isOutput=False)
    bbb = nc.declare_dram_parameter("bbb", [4, 1], F32, isOutput=False)

    out = nc.declare_dram_parameter("out", [4, OSP], F32, isOutput=True)

    # ---- internal DRAM ----
    tabs = [nc.dram_tensor("tab0", [OG, 64], BF16, addr_space="Shared")]
    for li in range(1, 4):
        tabs.append(nc.dram_tensor(f"tab{li}", [OG, P], BF16, addr_space="Shared"))
    preds = [pred0]
    for li in range(1, 4):
        preds.append(nc.dram_tensor(f"pred{li}", [P, T_PC], BF16))
    stages, sends, recvs, agins = [], [], [], []
    for li, (din, h, dout) in enumerate(DIMS):
        stages.append(nc.dram_tensor(f"stage{li}", [T_PC, h], BF16))
        sends.append(nc.dram_tensor(f"send{li}", [NC * S_B, h], BF16))
        recvs.append(nc.dram_tensor(f"recv{li}", [NC * S_B, h], BF16))
    agins.append(nc.dram_tensor("agin_e", [OSP, 64], BF16))
    for li in range(3):
        agins.append(nc.dram_tensor(f"agin{li}", [OSP, P], BF16))

    PRELU = mybir.ActivationFunctionType.Prelu
    COPY = mybir.ActivationFunctionType.Copy
    GRPS = [list(range(NC))]

    with tile.TileContext(nc) as tc:
        with tc.tile_pool(name="cst", bufs=1) as cst:
            # constants
            ident = cst.tile([P, P], F32)
            make_identity(nc, ident[:])
            ident_bf = cst.tile([P, P], BF16)
            nc.vector.tensor_copy(out=ident_bf[:], in_=ident[:])
            iota = cst.tile([P, P], I32)
            nc.gpsimd.iota(iota[:], pattern=[[1, P]], base=0, channel_multiplier=0)

            W = {}

            def load_w(name, src_ap, hh, ww, dt):
                t = cst.tile([hh, ww], dt, tag=name)
                nc.sync.dma_start(out=t[:], in_=src_ap)
                W[name] = t

            load_w("w_emb", w_emb[:, :], 68, 64, BF16)
            load_w("b_emb", b_emb[:, :], 64, 1, F32)
            load_w("wbb", wbb[:, :], P, 4, BF16)
            load_w("bbb", bbb[:, :], 4, 1, F32)
            load_w("invc", invc[:, :], P, NT, F32)
            for li, (din, h, dout) in enumerate(DIMS):
                for ki in range(3):
                    load_w(f"w1a{li}_c{ki}", wp[f"w1a{li}"][ki * din:(ki + 1) * din, :],
                           din, h, BF16)
                for k in range(h // P):
                    load_w(f"w1b{li}_{k}", wp[f"w1b{li}"][k * P:(k + 1) * P, :],
                           P, 2 * h + dout, BF16)
                    load_w(f"w2a{li}_{k}", wp[f"w2a{li}"][k * P:(k + 1) * P, :],
                           P, h, BF16)
                    load_w(f"w2b{li}_{k}", wp[f"w2b{li}"][k * P:(k + 1) * P, :],
                           P, dout, BF16)
                load_w(f"b1a{li}", wp[f"b1a{li}"][:, :], P, h // P, F32)
                load_w(f"b1bp{li}", wp[f"b1bp{li}"][:, :], dout, 1, F32)
                load_w(f"b1bs{li}", wp[f"b1bs{li}"][:, :], P, h, BF16)
                load_w(f"b1bo{li}", wp[f"b1bo{li}"][:, :], P, h, BF16)
                load_w(f"b2a{li}", wp[f"b2a{li}"][:, :], P, h // P, F32)
                load_w(f"b2b{li}", wp[f"b2b{li}"][:, :], dout, 1, F32)

            # ---------------- embedding phase ----------------
            NEB = -(-OSP // 512)
            with (
                tc.tile_pool(name="esb", bufs=3) as esb,
                tc.tile_pool(name="eps", bufs=3, space="PSUM") as eps,
            ):
                for b in range(NEB):
                    c0 = b * 512
                    w = min(512, OSP - c0)
                    xin = esb.tile([68, 512], BF16, tag="xin")
                    nc.sync.dma_start(out=xin[:, :w], in_=xt[:, c0:c0 + w])
                    pse = eps.tile([64, 512], F32, space="PSUM", tag="pse")
                    nc.tensor.matmul(out=pse[:, :w], lhsT=W["w_emb"][:], rhs=xin[:, :w],
                                     start=True, stop=True)
                    ebt = esb.tile([64, 512], BF16, tag="ebt")
                    nc.scalar.activation(out=ebt[:, :w], in_=pse[:, :w], func=PRELU,
                                         bias=W["b_emb"][:, :1], alpha=ALPHA)
                    for q in range(-(-w // P)):
                        qw = min(P, w - q * P)
                        ptr = eps.tile([P, 64], BF16, space="PSUM", tag="ptr")
                        nc.tensor.transpose(out=ptr[:qw, :], in_=ebt[:, q * P:q * P + qw],
                                            identity=ident_bf[:64, :64])
                        ent = esb.tile([P, 64], BF16, tag="ent")
                        nc.vector.tensor_copy(out=ent[:qw, :], in_=ptr[:qw, :])
                        nc.sync.dma_start(out=agins[0][c0 + q * P:c0 + q * P + qw, :],
                                          in_=ent[:qw, :])
            nc.gpsimd.collective_compute(
                "AllGather", mybir.AluOpType.bypass, replica_groups=GRPS,
                ins=[agins[0][:]], outs=[tabs[0][:]])

            # ---------------- layers ----------------
            _MAXL = int(os.environ.get("KMAXL", "4"))
            _NOPOOL = bool(int(os.environ.get("KNOPOOL", "0")))
            _NOMLP = bool(int(os.environ.get("KNOMLP", "0")))
            for li, (din, h, dout) in enumerate(DIMS[:_MAXL]):
                tab_in = tabs[li]
                pred_in = preds[li]
                stage, send, recv = stages[li], sends[li], recvs[li]
                NH = h // P
                s_cols = (0, h)
                p_cols = (h, h + dout)
                o_cols = (h + dout, 2 * h + dout)

                # ---- phase A: triple MLP ----
                if _NOMLP:
                    continue
                with (
                    tc.tile_pool(name=f"asb{li}", bufs=3) as asb,
                    tc.tile_pool(name=f"apstr{li}", bufs=2, space="PSUM") as aps_tr,
                    tc.tile_pool(name=f"apshid{li}", bufs=NH, space="PSUM") as aps_hid,
                    tc.tile_pool(name=f"apsout{li}", bufs=2, space="PSUM") as aps_out,
                ):
                    for j in range(NB):
                        sgi = asb.tile([P, 4], I32, tag="sgi")
                        nc.sync.dma_start(out=sgi[:], in_=sg[:, 4 * j:4 * j + 4])
                        ogi = asb.tile([P, 4], I32, tag="ogi")
                        nc.sync.dma_start(out=ogi[:], in_=og[:, 4 * j:4 * j + 4])
                        obi = asb.tile([P, 4], I32, tag="obi")
                        nc.sync.dma_start(out=obi[:], in_=ob[:, 4 * j:4 * j + 4])

                        sT = asb.tile([din, 512], BF16, tag="sT")
                        oT = asb.tile([din, 512], BF16, tag="oT")
                        for g in range(4):
                            for (idxt, dst) in ((sgi, sT), (ogi, oT)):
                                ge = asb.tile([P, din], BF16, tag="gath")
                                nc.gpsimd.indirect_dma_start(
                                    out=ge[:], out_offset=None, in_=tab_in[:],
                                    in_offset=bass.IndirectOffsetOnAxis(
                                        ap=idxt[:, g:g + 1], axis=0))
                                ptr = aps_tr.tile([din, P], BF16, space="PSUM", tag="ptr")
                                nc.tensor.transpose(out=ptr[:], in_=ge[:],
                                                    identity=ident_bf[:])
                                nc.vector.tensor_copy(out=dst[:, g * P:(g + 1) * P],
                                                      in_=ptr[:])
                        pT = asb.tile([din, 512], BF16, tag="pT")
                        nc.sync.dma_start(out=pT[:], in_=pred_in[:, 512 * j:512 * (j + 1)])

                        # hid
                        hidT = []
                        for mh in range(NH):
                            ph = aps_hid.tile([P, 512], F32, space="PSUM", tag="ph")
                            for ki, src in enumerate((sT, pT, oT)):
                                nc.tensor.matmul(
                                    out=ph[:],
                                    lhsT=W[f"w1a{li}_c{ki}"][:, mh * P:(mh + 1) * P],
                                    rhs=src[:],
                                    start=(ki == 0), stop=(ki == 2))
                            ht = asb.tile([P, 512], BF16, tag=f"hidT{mh}",
                                          name=f"hidT{mh}")
                            nc.scalar.activation(out=ht[:], in_=ph[:], func=PRELU,
                                                 bias=W[f"b1a{li}"][:, mh:mh + 1],
                                                 alpha=ALPHA)
                            hidT.append(ht)

                        # new_s / new_o (entry-major)
                        for (cols, bname, is_s) in ((s_cols, f"b1bs{li}", True),
                                                    (o_cols, f"b1bo{li}", False)):
                            for e in range(4):
                                po = aps_out.tile([P, 512], F32, space="PSUM", tag="po")
                                for k in range(NH):
                                    nc.tensor.matmul(
                                        out=po[:, :h],
                                        lhsT=hidT[k][:, e * P:(e + 1) * P],
                                        rhs=W[f"w1b{li}_{k}"][:, cols[0]:cols[1]],
                                        start=(k == 0), stop=(k == NH - 1))
                                nc.vector.tensor_tensor(
                                    out=po[:, :h], in0=po[:, :h], in1=W[bname][:],
                                    op=mybir.AluOpType.add)
                                ov = asb.tile([P, 512], BF16, tag="ov")
                                nc.scalar.activation(out=ov[:, :h], in_=po[:, :h],
                                                     func=PRELU, alpha=ALPHA)
                                if is_s:
                                    r0 = 512 * j + e * P
                                    nc.sync.dma_start(out=stage[r0:r0 + P, :],
                                                      in_=ov[:, :h])
                                else:
                                    nc.gpsimd.indirect_dma_start(
                                        out=send[:],
                                        out_offset=bass.IndirectOffsetOnAxis(
                                            ap=obi[:, e:e + 1], axis=0),
                                        in_=ov[:, :h], in_offset=None)

                        # new_p (feature-major), not needed after last layer
                        if li < 3:
                            pp = aps_out.tile([P, 512], F32, space="PSUM", tag="po")
                            for k in range(NH):
                                nc.tensor.matmul(
                                    out=pp[:dout, :],
                                    lhsT=W[f"w1b{li}_{k}"][:, p_cols[0]:p_cols[1]],
                                    rhs=hidT[k][:],
                                    start=(k == 0), stop=(k == NH - 1))
                            pv = asb.tile([dout, 512], BF16, tag="pv")
                            nc.scalar.activation(out=pv[:], in_=pp[:dout, :], func=PRELU,
                                                 bias=W[f"b1bp{li}"][:, :1], alpha=ALPHA)
                            nc.sync.dma_start(
                                out=preds[li + 1][:, 512 * j:512 * (j + 1)], in_=pv[:])

                # ---- phase B: AllToAll ----
                nc.gpsimd.collective_compute(
                    "AllToAll", mybir.AluOpType.bypass, replica_groups=GRPS,
                    ins=[send[:]], outs=[recv[:]])

                # ---- phase C: pooling + object MLP ----
                if _NOPOOL:
                    continue
                with (
                    tc.tile_pool(name=f"csb{li}", bufs=3) as csb,
                    tc.tile_pool(name=f"cpool{li}", bufs=2, space="PSUM") as cps_pool,
                    tc.tile_pool(name=f"ctr{li}", bufs=2, space="PSUM") as cps_tr,
                    tc.tile_pool(name=f"cmlp{li}", bufs=2, space="PSUM") as cps_mlp,
                ):
                    ng = -(-NT // 4)
                    for grp in range(ng):
                        t0 = grp * 4
                        tn = min(4, NT - t0)
                        gw = tn * P
                        pooledT = [csb.tile([P, 512], BF16, tag=f"pooledT{k}",
                                            name=f"pooledT{k}")
                                   for k in range(NH)]
                        for tt in range(t0, t0 + tn):
                            pps = cps_pool.tile([P, h], F32, space="PSUM", tag="pps")
                            nmm = 0
                            for (ilp, PN, src) in ((spil, PS, stage), (opil, PO, recv)):
                                for k in range(PN):
                                    col = (tt * PN + k) * 2
                                    ilt = csb.tile([P, 2], I32, tag="ilt")
                                    nc.sync.dma_start(out=ilt[:], in_=ilp[:, col:col + 2])
                                    vals = csb.tile([P, h], BF16, tag="vals")
                                    nc.gpsimd.indirect_dma_start(
                                        out=vals[:], out_offset=None, in_=src[:],
                                        in_offset=bass.IndirectOffsetOnAxis(
                                            ap=ilt[:, 0:1], axis=0))
                                    oh = csb.tile([P, P], BF16, tag="oh")
                                    nc.vector.tensor_tensor(
                                        out=oh[:], in0=ilt[:, 1:2].to_broadcast([P, P]),
                                        in1=iota[:], op=mybir.AluOpType.is_equal)
                                    nc.tensor.matmul(out=pps[:], lhsT=oh[:], rhs=vals[:],
                                                     start=(nmm == 0),
                                                     stop=(nmm == (PS + PO) - 1))
                                    nmm += 1
                            pob = csb.tile([P, h], BF16, tag="pob")
                            nc.scalar.activation(out=pob[:], in_=pps[:], func=COPY,
                                                 scale=W["invc"][:, tt:tt + 1])
                            for k in range(NH):
                                ptr2 = cps_tr.tile([P, P], BF16, space="PSUM", tag="ptr2")
                                nc.tensor.transpose(out=ptr2[:],
                                                    in_=pob[:, k * P:(k + 1) * P],
                                                    identity=ident_bf[:])
                                nc.vector.tensor_copy(
                                    out=pooledT[k][:, (tt - t0) * P:(tt - t0 + 1) * P],
                                    in_=ptr2[:])
                        # object MLP on gw objects
                        hid2 = []
                        for mh in range(NH):
                            p2 = cps_mlp.tile([P, 512], F32, space="PSUM", tag="p2")
                            for k in range(NH):
                                nc.tensor.matmul(
                                    out=p2[:, :gw],
                                    lhsT=W[f"w2a{li}_{k}"][:, mh * P:(mh + 1) * P],
                                    rhs=pooledT[k][:, :gw],
                                    start=(k == 0), stop=(k == NH - 1))
                            h2 = csb.tile([P, 512], BF16, tag=f"h2_{mh}",
                                          name=f"h2_{mh}")
                            nc.scalar.activation(out=h2[:, :gw], in_=p2[:, :gw],
                                                 func=PRELU,
                                                 bias=W[f"b2a{li}"][:, mh:mh + 1],
                                                 alpha=ALPHA)
                            hid2.append(h2)
                        pno = cps_mlp.tile([P, 512], F32, space="PSUM", tag="p2")
                        for k in range(NH):
                            nc.tensor.matmul(out=pno[:dout, :gw],
                                             lhsT=W[f"w2b{li}_{k}"][:],
                                             rhs=hid2[k][:, :gw],
                                             start=(k == 0), stop=(k == NH - 1))
                        noT = csb.tile([dout, 512], BF16, tag="noT")
                        nc.scalar.activation(out=noT[:, :gw], in_=pno[:dout, :gw],
                                             func=PRELU, bias=W[f"b2b{li}"][:, :1],
                                             alpha=ALPHA)
                        if li < 3:
                            for q in range(tn):
                                ptr3 = cps_tr.tile([P, P], BF16, space="PSUM", tag="ptr2")
                                nc.tensor.transpose(out=ptr3[:, :dout],
                                                    in_=noT[:, q * P:(q + 1) * P],
                                                    identity=ident_bf[:])
                                ent2 = csb.tile([P, P], BF16, tag="ent2")
                                nc.vector.tensor_copy(out=ent2[:, :dout],
                                                      in_=ptr3[:, :dout])
                                r0 = (t0 + q) * P
                                nc.sync.dma_start(out=agins[li + 1][r0:r0 + P, :],
                                                  in_=ent2[:, :dout])
                        else:
                            phd = cps_mlp.tile([4, 512], F32, space="PSUM", tag="phd")
                            nc.tensor.matmul(out=phd[:, :gw], lhsT=W["wbb"][:],
                                             rhs=noT[:, :gw], start=True, stop=True)
                            ho = csb.tile([4, 512], F32, tag="ho")
                            nc.scalar.activation(out=ho[:, :gw], in_=phd[:, :gw],
                                                 func=PRELU, bias=W["bbb"][:, :1],
                                                 alpha=ALPHA)
                            nc.sync.dma_start(out=out[:, t0 * P:t0 * P + gw],
                                              in_=ho[:, :gw])

                # ---- phase D: AllGather new object table ----
                if li < 3:
                    nc.gpsimd.collective_compute(
                        "AllGather", mybir.AluOpType.bypass, replica_groups=GRPS,
                        ins=[agins[li + 1][:]], outs=[tabs[li + 1][:]])

    nc.compile()
    return nc


# ---------------------------------------------------------------------------
# Entry point
# ---------------------------------------------------------------------------

_CACHE = {}


def kernel(**inputs) -> np.ndarray:
    cfg, in_maps = preprocess(inputs)
    key = tuple(sorted(cfg.items()))
    if key not in _CACHE:
        _CACHE[key] = build_kernel(cfg)
    nc = _CACHE[key]
    res = run_bass_kernel_spmd(nc, in_maps, list(range(NC)))
    O, OS = cfg["O"], cfg["OS"]
    full = np.zeros((4, O), np.float32)
    for c in range(NC):
        full[:, c * OS:(c + 1) * OS] = res.results[c]["out"][:, :OS]
    return np.ascontiguousarray(full.T)



# revision 4
# speedup vs baseline: 8.9950x; 8.9950x over previous
"""Trainium2 Bass kernel for nn_NDNRefinement (4-layer GNN message passing).

Strategy (8 NeuronCores), V2:
- Shard triples by subject core (s // OS). Within a core, triples are sorted
  by (dest core of object, o_local) and padded so each destination section is
  exactly S_B entries. new_o rows are then written CONTIGUOUSLY into the
  AllToAll send buffer (no scatter); row indices all fit int16.
- All gathers use single batched dma_gather instructions:
  * phase A inputs: transpose-gather (feature-major) of subject rows from the
    local object table and object rows from the replicated global table (two
    overlapping windows handle >32k rows; out-of-window entries hit dedicated
    zero rows and the two partial gathers accumulate in the matmul).
  * pooling: entry-major 768-row gathers from stage/recv per object tile,
    pooled with one-hot matmuls (iota + is_equal).
- Index schedules are resident in SBUF (loaded once, shared by all layers).
- Per-object MLPs are data-parallel over the object shard; the new object
  table is AllGathered between layers into a window-padded table.
"""

import os
import numpy as np
import ml_dtypes

import concourse.bass as bass
import concourse.bacc as bacc
import concourse.tile as tile
from concourse import mybir
from concourse import library_config
from concourse.bass_utils import run_bass_kernel_spmd
from concourse.masks import make_identity

BF16 = mybir.dt.bfloat16
F32 = mybir.dt.float32
I32 = mybir.dt.int32
I16 = mybir.dt.int16
P = 128
NC = 8
ALPHA = 0.2
DF = 128                       # object-table feature width (padded)
IDXCAP = 32768                 # int16 index window size

# (din, h, dout) per layer
DIMS = [(64, 512, 128), (128, 512, 128), (128, 512, 128), (128, 128, 128)]


def _rup(x, m):
    return ((int(x) + m - 1) // m) * m


def _pack16(vals, total):
    """int16 index list -> [128, total//16] wrapped tile (j -> [j%16, j//16]),
    replicated to 128 partitions."""
    a = np.zeros((total,), np.int16)
    a[:len(vals)] = vals
    w = a.reshape(total // 16, 16).T          # [16, total//16]
    return np.tile(w, (8, 1)).copy()          # [128, total//16]


# ---------------------------------------------------------------------------
# Host preprocessing
# ---------------------------------------------------------------------------

def preprocess(inputs):
    obj_vecs = np.asarray(inputs["obj_vecs"], np.float32)
    pred_vecs = np.asarray(inputs["pred_vecs"], np.float32)
    pred_boxes = np.asarray(inputs["pred_boxes"], np.float32)
    s_idx = np.asarray(inputs["s_idx"], np.int32)
    o_idx = np.asarray(inputs["o_idx"], np.int32)

    O = obj_vecs.shape[0]
    assert O % NC == 0
    OS = O // NC
    OSP = _rup(OS, P)
    NT = OSP // P
    OG = NC * OSP
    TABR = OG + 2 * P            # front + tail zero rows
    WB0 = max(0, TABR - IDXCAP)  # window B start row
    ZBI = OG + P - WB0           # window-B index of a tail zero row

    core = s_idx // OS
    percore = []
    S_B = 0
    for c in range(NC):
        idxs = np.where(core == c)[0]
        d = o_idx[idxs] // OS
        o_loc = o_idx[idxs] % OS
        order = np.lexsort((o_loc, d))
        idxs = idxs[order]
        d = d[order]
        o_loc = o_loc[order]
        bc = np.bincount(d, minlength=NC)
        S_B = max(S_B, int(bc.max()))
        percore.append(dict(idxs=idxs, d=d, o_loc=o_loc, bc=bc,
                            s_loc=s_idx[idxs] - c * OS))
    NCSB = NC * S_B
    assert NCSB <= IDXCAP, f"NCSB={NCSB} exceeds int16 window"
    TP = _rup(NCSB, 512)
    NB = TP // 512

    # per-object counts
    cnt = np.bincount(s_idx, minlength=O) + np.bincount(o_idx, minlength=O)
    inv_cnt = (1.0 / np.maximum(cnt, 1)).astype(np.float32)

    # entry position arrays per core
    pos_of = []           # per core: position of each (sorted) real triple
    for c in range(NC):
        pc = percore[c]
        first = np.concatenate([[0], np.cumsum(pc["bc"])[:-1]])
        rank = np.arange(len(pc["d"])) - first[pc["d"]]
        pos = (pc["d"] * S_B + rank).astype(np.int64)
        pos_of.append(pos)

    # max pooling loads
    max_s_load = 1
    max_o_load = 1
    for c in range(NC):
        pc = percore[c]
        sload = np.bincount(pc["s_loc"] // P, minlength=NT)
        max_s_load = max(max_s_load, int(sload.max()))
    for c in range(NC):
        oload = np.zeros((NT,), np.int64)
        for sc in range(NC):
            pc = percore[sc]
            m = pc["d"] == c
            oload += np.bincount(pc["o_loc"][m] // P, minlength=NT)
        max_o_load = max(max_o_load, int(oload.max()))
    PS = -(-max_s_load // P)
    PO = -(-max_o_load // P)

    cfg = dict(O=O, OS=OS, OSP=OSP, NT=NT, OG=OG, TABR=TABR, WB0=WB0,
               S_B=S_B, NCSB=NCSB, TP=TP, NB=NB, PS=PS, PO=PO)

    # ---- weights, shared across cores ----
    bf = ml_dtypes.bfloat16
    shared = {}
    w_emb = np.zeros((68, DF), np.float32)
    w_emb[:, :64] = np.asarray(inputs["W_emb"], np.float32)
    shared["w_emb"] = w_emb.astype(bf)
    b_emb = np.zeros((DF, 1), np.float32)
    b_emb[:64, 0] = np.asarray(inputs["b_emb"], np.float32)
    shared["b_emb"] = b_emb
    for li, (din, h, dout) in enumerate(DIMS):
        b1b = np.asarray(inputs[f"b1b{li}"], np.float32)
        shared[f"w1a{li}"] = np.asarray(inputs[f"W1a{li}"], np.float32).astype(bf)
        shared[f"w1b{li}"] = np.asarray(inputs[f"W1b{li}"], np.float32).astype(bf)
        shared[f"w2a{li}"] = np.asarray(inputs[f"W2a{li}"], np.float32).astype(bf)
        shared[f"w2b{li}"] = np.asarray(inputs[f"W2b{li}"], np.float32).astype(bf)
        shared[f"b1a{li}"] = np.asarray(inputs[f"b1a{li}"], np.float32).reshape(-1, P).T.copy()
        shared[f"b1bp{li}"] = b1b[h:h + dout].reshape(-1, 1).copy()
        shared[f"b1bs{li}"] = np.broadcast_to(b1b[:h].astype(bf), (P, h)).copy()
        shared[f"b1bo{li}"] = np.broadcast_to(b1b[h + dout:].astype(bf), (P, h)).copy()
        shared[f"b2a{li}"] = np.asarray(inputs[f"b2a{li}"], np.float32).reshape(-1, P).T.copy()
        shared[f"b2b{li}"] = np.asarray(inputs[f"b2b{li}"], np.float32).reshape(-1, 1).copy()
    shared["wbb"] = np.asarray(inputs["W_bb"], np.float32).astype(bf)
    shared["bbb"] = np.asarray(inputs["b_bb"], np.float32).reshape(-1, 1)

    x_full = np.concatenate([obj_vecs, pred_boxes], axis=1)      # (O, 68)

    in_maps = []
    for c in range(NC):
        pc = percore[c]
        pos = pos_of[c]
        n = len(pos)
        m = {}
        xT = np.zeros((68, OSP), bf)
        xT[:, :OS] = x_full[c * OS:(c + 1) * OS].T.astype(bf)
        m["xt"] = xT

        pT = np.zeros((64, TP), bf)
        pT[:, pos] = pred_vecs[pc["idxs"]].T.astype(bf)
        m["pred0"] = pT

        # per-entry gather indices (by position)
        ent_s = np.full((TP,), OSP, np.int16)
        ent_s[pos] = pc["s_loc"].astype(np.int16)
        g = (pc["d"] * OSP + pc["o_loc"]).astype(np.int64)
        ent_oa = np.zeros((TP,), np.int16)
        ent_ob = np.full((TP,), ZBI, np.int16)
        loA = g <= IDXCAP - 1 - P
        ent_oa[pos[loA]] = (g[loA] + P).astype(np.int16)
        ent_oa[pos[~loA]] = 0
        ent_ob[pos[~loA]] = (g[~loA] + P - WB0).astype(np.int16)
        m["sidx"] = _pack16(ent_s, TP)
        m["oaidx"] = _pack16(ent_oa, TP)
        m["obidx"] = _pack16(ent_ob, TP)

        # s-pool schedule: per tile, positions of entries with s in tile
        sp16 = np.zeros((NT, PS * P), np.int16)
        spl = np.full((NT, PS * P), -1, np.int32)
        tsel = pc["s_loc"] // P
        for t in range(NT):
            mask = tsel == t
            k = int(mask.sum())
            sp16[t, :k] = pos[mask].astype(np.int16)
            spl[t, :k] = (pc["s_loc"][mask] - t * P).astype(np.int32)
        m["spool"] = np.concatenate(
            [_pack16(sp16[t], PS * P) for t in range(NT)], axis=1)
        m["sloc"] = spl.reshape(NT * PS, P).T.copy()

        # o-pool schedule: recv rows for my tiles
        rows_all, locs_all = [], []
        for sc in range(NC):
            qc = percore[sc]
            mm = qc["d"] == c
            rows_all.append(pos_of[sc][mm] - c * S_B + sc * S_B)
            locs_all.append(qc["o_loc"][mm])
        rows_all = np.concatenate(rows_all)
        locs_all = np.concatenate(locs_all)
        op16 = np.zeros((NT, PO * P), np.int16)
        opl = np.full((NT, PO * P), -1, np.int32)
        tsel = locs_all // P
        for t in range(NT):
            mask = tsel == t
            k = int(mask.sum())
            op16[t, :k] = rows_all[mask].astype(np.int16)
            opl[t, :k] = (locs_all[mask] - t * P).astype(np.int32)
        m["opool"] = np.concatenate(
            [_pack16(op16[t], PO * P) for t in range(NT)], axis=1)
        m["oloc"] = opl.reshape(NT * PO, P).T.copy()

        iv = np.zeros((OSP,), np.float32)
        iv[:OS] = inv_cnt[c * OS:(c + 1) * OS]
        m["invc"] = iv.reshape(NT, P).T.copy()
        m.update(shared)
        in_maps.append(m)

    return cfg, in_maps


# ---------------------------------------------------------------------------
# Kernel builder
# ---------------------------------------------------------------------------

def build_kernel(cfg):
    OSP, NT, OG, TABR, WB0 = (cfg["OSP"], cfg["NT"], cfg["OG"], cfg["TABR"],
                              cfg["WB0"])
    NCSB, TP, NB, PS, PO = (cfg["NCSB"], cfg["TP"], cfg["NB"], cfg["PS"],
                            cfg["PO"])
    WA = min(IDXCAP, TABR)       # window A size
    WBN = TABR - WB0             # window B size
    SW = OSP + P                 # local (subject) window size

    nc = bacc.Bacc("TRN2", target_bir_lowering=False, debug=False,
                   num_devices=NC)

    # ---- parameters ----
    xt = nc.declare_dram_parameter("xt", [68, OSP], BF16, isOutput=False)
    pred0 = nc.declare_dram_parameter("pred0", [64, TP], BF16, isOutput=False)
    sidx = nc.declare_dram_parameter("sidx", [P, TP // 16], I16, isOutput=False)
    oaidx = nc.declare_dram_parameter("oaidx", [P, TP // 16], I16, isOutput=False)
    obidx = nc.declare_dram_parameter("obidx", [P, TP // 16], I16, isOutput=False)
    spool = nc.declare_dram_parameter("spool", [P, NT * PS * 8], I16, isOutput=False)
    opool = nc.declare_dram_parameter("opool", [P, NT * PO * 8], I16, isOutput=False)
    sloc = nc.declare_dram_parameter("sloc", [P, NT * PS], I32, isOutput=False)
    oloc = nc.declare_dram_parameter("oloc", [P, NT * PO], I32, isOutput=False)
    invc = nc.declare_dram_parameter("invc", [P, NT], F32, isOutput=False)

    w_emb = nc.declare_dram_parameter("w_emb", [68, DF], BF16, isOutput=False)
    b_emb = nc.declare_dram_parameter("b_emb", [DF, 1], F32, isOutput=False)
    wp = {}
    for li, (din, h, dout) in enumerate(DIMS):
        wp[f"w1a{li}"] = nc.declare_dram_parameter(f"w1a{li}", [3 * din, h], BF16, isOutput=False)
        wp[f"w1b{li}"] = nc.declare_dram_parameter(f"w1b{li}", [h, 2 * h + dout], BF16, isOutput=False)
        wp[f"w2a{li}"] = nc.declare_dram_parameter(f"w2a{li}", [h, h], BF16, isOutput=False)
        wp[f"w2b{li}"] = nc.declare_dram_parameter(f"w2b{li}", [h, dout], BF16, isOutput=False)
        wp[f"b1a{li}"] = nc.declare_dram_parameter(f"b1a{li}", [P, h // P], F32, isOutput=False)
        wp[f"b1bp{li}"] = nc.declare_dram_parameter(f"b1bp{li}", [dout, 1], F32, isOutput=False)
        wp[f"b1bs{li}"] = nc.declare_dram_parameter(f"b1bs{li}", [P, h], BF16, isOutput=False)
        wp[f"b1bo{li}"] = nc.declare_dram_parameter(f"b1bo{li}", [P, h], BF16, isOutput=False)
        wp[f"b2a{li}"] = nc.declare_dram_parameter(f"b2a{li}", [P, h // P], F32, isOutput=False)
        wp[f"b2b{li}"] = nc.declare_dram_parameter(f"b2b{li}", [dout, 1], F32, isOutput=False)
    wbb = nc.declare_dram_parameter("wbb", [P, 4], BF16, isOutput=False)
    bbb = nc.declare_dram_parameter("bbb", [4, 1], F32, isOutput=False)

    out = nc.declare_dram_parameter("out", [4, OSP], F32, isOutput=True)

    # ---- internal DRAM ----
    tabs, agins = [], []
    for li in range(4):
        tabs.append(nc.dram_tensor(f"tab{li}", [TABR, DF], BF16,
                                   addr_space="Shared"))
        agins.append(nc.dram_tensor(f"agin{li}", [OSP + P, DF], BF16))
    preds = [pred0]
    for li in range(1, 4):
        preds.append(nc.dram_tensor(f"pred{li}", [P, TP], BF16))
    stages, sends, recvs = [], [], []
    for li, (din, h, dout) in enumerate(DIMS):
        stages.append(nc.dram_tensor(f"stage{li}", [TP, h], BF16))
        sends.append(nc.dram_tensor(f"send{li}", [TP, h], BF16))
        recvs.append(nc.dram_tensor(f"recv{li}", [NCSB, h], BF16))

    PRELU = mybir.ActivationFunctionType.Prelu
    COPY = mybir.ActivationFunctionType.Copy
    GRPS = [list(range(NC))]

    with tile.TileContext(nc) as tc:
        with tc.tile_pool(name="cst", bufs=1) as cst:
            # constants (iota needs the standard gpsimd library: do it first)
            ident = cst.tile([P, P], F32)
            make_identity(nc, ident[:])
            ident_bf = cst.tile([P, P], BF16)
            nc.vector.tensor_copy(out=ident_bf[:], in_=ident[:])
            iota = cst.tile([P, P], I32)
            nc.gpsimd.iota(iota[:], pattern=[[1, P]], base=0,
                           channel_multiplier=0)
            zt = cst.tile([P, DF], BF16)
            nc.gpsimd.memset(zt[:], 0)
            nc.gpsimd.load_library(library_config.mlp)

            # zero rows of the windowed tables (once per call)
            for li in range(4):
                nc.sync.dma_start(out=tabs[li][0:P, :], in_=zt[:])
                nc.sync.dma_start(out=tabs[li][OG + P:OG + 2 * P, :], in_=zt[:])
                nc.sync.dma_start(out=agins[li][OSP:OSP + P, :], in_=zt[:])

            W = {}

            def load_w(name, src_ap, hh, ww, dt):
                t = cst.tile([hh, ww], dt, tag=name)
                nc.sync.dma_start(out=t[:], in_=src_ap)
                W[name] = t

            load_w("w_emb", w_emb[:, :], 68, DF, BF16)
            load_w("b_emb", b_emb[:, :], DF, 1, F32)
            load_w("wbb", wbb[:, :], P, 4, BF16)
            load_w("bbb", bbb[:, :], 4, 1, F32)
            load_w("invc", invc[:, :], P, NT, F32)
            load_w("sidx", sidx[:, :], P, TP // 16, I16)
            load_w("oaidx", oaidx[:, :], P, TP // 16, I16)
            load_w("obidx", obidx[:, :], P, TP // 16, I16)
            load_w("spool", spool[:, :], P, NT * PS * 8, I16)
            load_w("opool", opool[:, :], P, NT * PO * 8, I16)
            load_w("sloc", sloc[:, :], P, NT * PS, I32)
            load_w("oloc", oloc[:, :], P, NT * PO, I32)
            for li, (din, h, dout) in enumerate(DIMS):
                for ki in range(3):
                    load_w(f"w1a{li}_c{ki}",
                           wp[f"w1a{li}"][ki * din:(ki + 1) * din, :],
                           din, h, BF16)
                for k in range(h // P):
                    load_w(f"w1b{li}_{k}", wp[f"w1b{li}"][k * P:(k + 1) * P, :],
                           P, 2 * h + dout, BF16)
                    load_w(f"w2a{li}_{k}", wp[f"w2a{li}"][k * P:(k + 1) * P, :],
                           P, h, BF16)
                    load_w(f"w2b{li}_{k}", wp[f"w2b{li}"][k * P:(k + 1) * P, :],
                           P, dout, BF16)
                load_w(f"b1a{li}", wp[f"b1a{li}"][:, :], P, h // P, F32)
                load_w(f"b1bp{li}", wp[f"b1bp{li}"][:, :], dout, 1, F32)
                load_w(f"b1bs{li}", wp[f"b1bs{li}"][:, :], P, h, BF16)
                load_w(f"b1bo{li}", wp[f"b1bo{li}"][:, :], P, h, BF16)
                load_w(f"b2a{li}", wp[f"b2a{li}"][:, :], P, h // P, F32)
                load_w(f"b2b{li}", wp[f"b2b{li}"][:, :], dout, 1, F32)

            # ---------------- embedding phase ----------------
            NEB = -(-OSP // 512)
            with (
                tc.tile_pool(name="esb", bufs=3) as esb,
                tc.tile_pool(name="eps", bufs=3, space="PSUM") as eps,
            ):
                for b in range(NEB):
                    c0 = b * 512
                    w = min(512, OSP - c0)
                    xin = esb.tile([68, 512], BF16, tag="xin")
                    nc.sync.dma_start(out=xin[:, :w], in_=xt[:, c0:c0 + w])
                    pse = eps.tile([DF, 512], F32, space="PSUM", tag="pse")
                    nc.tensor.matmul(out=pse[:, :w], lhsT=W["w_emb"][:],
                                     rhs=xin[:, :w], start=True, stop=True)
                    ebt = esb.tile([DF, 512], BF16, tag="ebt")
                    nc.scalar.activation(out=ebt[:, :w], in_=pse[:, :w],
                                         func=PRELU, bias=W["b_emb"][:, :1],
                                         alpha=ALPHA)
                    for q in range(-(-w // P)):
                        qw = min(P, w - q * P)
                        ptr = eps.tile([P, DF], BF16, space="PSUM", tag="ptr")
                        nc.tensor.transpose(out=ptr[:qw, :],
                                            in_=ebt[:, q * P:q * P + qw],
                                            identity=ident_bf[:])
                        ent = esb.tile([P, DF], BF16, tag="ent")
                        nc.vector.tensor_copy(out=ent[:qw, :], in_=ptr[:qw, :])
                        nc.sync.dma_start(
                            out=agins[0][c0 + q * P:c0 + q * P + qw, :],
                            in_=ent[:qw, :])
            nc.gpsimd.collective_compute(
                "AllGather", mybir.AluOpType.bypass, replica_groups=GRPS,
                ins=[agins[0][0:OSP, :]], outs=[tabs[0][P:P + OG, :]])

            # ---------------- layers ----------------
            _MAXL = int(os.environ.get("KMAXL", "4"))
            for li, (din, h, dout) in enumerate(DIMS[:_MAXL]):
                tab_in, agin_in = tabs[li], agins[li]
                pred_in = preds[li]
                stage, send, recv = stages[li], sends[li], recvs[li]
                NH = h // P
                s_cols = (0, h)
                p_cols = (h, h + dout)
                o_cols = (h + dout, 2 * h + dout)

                # ---- phase A: triple MLP ----
                with (
                    tc.tile_pool(name=f"asb{li}", bufs=3) as asb,
                    tc.tile_pool(name=f"apshid{li}", bufs=NH, space="PSUM") as aps_hid,
                    tc.tile_pool(name=f"apsout{li}", bufs=2, space="PSUM") as aps_out,
                ):
                    for j in range(NB):
                        ic0 = j * 32
                        sT = asb.tile([P, 1, 512], BF16, tag="sT")
                        nc.gpsimd.dma_gather(
                            sT[:], agin_in[0:SW, :],
                            W["sidx"][:, ic0:ic0 + 32], 512, 512, DF,
                            transpose=True)
                        oTA = asb.tile([P, 1, 512], BF16, tag="oTA")
                        nc.gpsimd.dma_gather(
                            oTA[:], tab_in[0:WA, :],
                            W["oaidx"][:, ic0:ic0 + 32], 512, 512, DF,
                            transpose=True)
                        oTB = asb.tile([P, 1, 512], BF16, tag="oTB")
                        nc.gpsimd.dma_gather(
                            oTB[:], tab_in[WB0:WB0 + WBN, :],
                            W["obidx"][:, ic0:ic0 + 32], 512, 512, DF,
                            transpose=True)
                        oT = asb.tile([P, 512], BF16, tag="oT")
                        nc.vector.tensor_tensor(
                            out=oT[:], in0=oTA[:, 0, :], in1=oTB[:, 0, :],
                            op=mybir.AluOpType.add)
                        pT = asb.tile([din, 512], BF16, tag="pT")
                        nc.sync.dma_start(out=pT[:],
                                          in_=pred_in[:din, 512 * j:512 * (j + 1)])

                        # hid
                        hidT = []
                        for mh in range(NH):
                            ph = aps_hid.tile([P, 512], F32, space="PSUM", tag="ph")
                            for ki, src in enumerate(
                                    (sT[:din, 0, :], pT[:], oT[:din, :])):
                                nc.tensor.matmul(
                                    out=ph[:],
                                    lhsT=W[f"w1a{li}_c{ki}"][:, mh * P:(mh + 1) * P],
                                    rhs=src,
                                    start=(ki == 0), stop=(ki == 2))
                            ht = asb.tile([P, 512], BF16, tag=f"hidT{mh}",
                                          name=f"hidT{mh}")
                            nc.scalar.activation(out=ht[:], in_=ph[:], func=PRELU,
                                                 bias=W[f"b1a{li}"][:, mh:mh + 1],
                                                 alpha=ALPHA)
                            hidT.append(ht)

                        # new_s / new_o (entry-major, contiguous rows)
                        for (cols, bname, dst) in ((s_cols, f"b1bs{li}", stage),
                                                   (o_cols, f"b1bo{li}", send)):
                            for e in range(4):
                                po = aps_out.tile([P, 512], F32, space="PSUM", tag="po")
                                for k in range(NH):
                                    nc.tensor.matmul(
                                        out=po[:, :h],
                                        lhsT=hidT[k][:, e * P:(e + 1) * P],
                                        rhs=W[f"w1b{li}_{k}"][:, cols[0]:cols[1]],
                                        start=(k == 0), stop=(k == NH - 1))
                                nc.vector.tensor_tensor(
                                    out=po[:, :h], in0=po[:, :h], in1=W[bname][:],
                                    op=mybir.AluOpType.add)
                                ov = asb.tile([P, 512], BF16, tag="ov")
                                nc.scalar.activation(out=ov[:, :h], in_=po[:, :h],
                                                     func=PRELU, alpha=ALPHA)
                                r0 = 512 * j + e * P
                                nc.sync.dma_start(out=dst[r0:r0 + P, :],
                                                  in_=ov[:, :h])

                        # new_p (feature-major), not needed after last layer
                        if li < 3:
                            pp = aps_out.tile([P, 512], F32, space="PSUM", tag="po")
                            for k in range(NH):
                                nc.tensor.matmul(
                                    out=pp[:dout, :],
                                    lhsT=W[f"w1b{li}_{k}"][:, p_cols[0]:p_cols[1]],
                                    rhs=hidT[k][:],
                                    start=(k == 0), stop=(k == NH - 1))
                            pv = asb.tile([dout, 512], BF16, tag="pv")
                            nc.scalar.activation(out=pv[:], in_=pp[:dout, :],
                                                 func=PRELU,
                                                 bias=W[f"b1bp{li}"][:, :1],
                                                 alpha=ALPHA)
                            nc.sync.dma_start(
                                out=preds[li + 1][:, 512 * j:512 * (j + 1)],
                                in_=pv[:])

                # ---- phase B: AllToAll ----
                nc.gpsimd.collective_compute(
                    "AllToAll", mybir.AluOpType.bypass, replica_groups=GRPS,
                    ins=[send[0:NCSB, :]], outs=[recv[:]])

                # ---- phase C: pooling + object MLP ----
                with (
                    tc.tile_pool(name=f"cgth{li}", bufs=2) as cgth,
                    tc.tile_pool(name=f"csb{li}", bufs=3) as csb,
                    tc.tile_pool(name=f"cpool{li}", bufs=2, space="PSUM") as cps_pool,
                    tc.tile_pool(name=f"ctr{li}", bufs=2, space="PSUM") as cps_tr,
                    tc.tile_pool(name=f"cmlp{li}", bufs=2, space="PSUM") as cps_mlp,
                ):
                    ng = -(-NT // 4)
                    for grp in range(ng):
                        t0 = grp * 4
                        tn = min(4, NT - t0)
                        gw = tn * P
                        pooledT = [csb.tile([P, 512], BF16, tag=f"pooledT{k}",
                                            name=f"pooledT{k}")
                                   for k in range(NH)]
                        for tt in range(t0, t0 + tn):
                            vs = cgth.tile([P, PS, h], BF16, tag="vs")
                            nc.gpsimd.dma_gather(
                                vs[:], stage[:, :],
                                W["spool"][:, tt * PS * 8:(tt + 1) * PS * 8],
                                PS * P, PS * P, h)
                            vo = cgth.tile([P, PO, h], BF16, tag="vo")
                            nc.gpsimd.dma_gather(
                                vo[:], recv[:, :],
                                W["opool"][:, tt * PO * 8:(tt + 1) * PO * 8],
                                PO * P, PO * P, h)
                            pps = cps_pool.tile([P, h], F32, space="PSUM", tag="pps")
                            nmm = 0
                            for (vals, locs, PN) in ((vs, "sloc", PS),
                                                     (vo, "oloc", PO)):
                                for k in range(PN):
                                    col = tt * PN + k
                                    oh = csb.tile([P, P], BF16, tag="oh")
                                    nc.vector.tensor_tensor(
                                        out=oh[:],
                                        in0=W[locs][:, col:col + 1].to_broadcast([P, P]),
                                        in1=iota[:], op=mybir.AluOpType.is_equal)
                                    nc.tensor.matmul(out=pps[:], lhsT=oh[:],
                                                     rhs=vals[:, k, :],
                                                     start=(nmm == 0),
                                                     stop=(nmm == (PS + PO) - 1))
                                    nmm += 1
                            pob = csb.tile([P, h], BF16, tag="pob")
                            nc.scalar.activation(out=pob[:], in_=pps[:], func=COPY,
                                                 scale=W["invc"][:, tt:tt + 1])
                            for k in range(NH):
                                ptr2 = cps_tr.tile([P, P], BF16, space="PSUM", tag="ptr2")
                                nc.tensor.transpose(out=ptr2[:],
                                                    in_=pob[:, k * P:(k + 1) * P],
                                                    identity=ident_bf[:])
                                nc.vector.tensor_copy(
                                    out=pooledT[k][:, (tt - t0) * P:(tt - t0 + 1) * P],
                                    in_=ptr2[:])
                        # object MLP on gw objects
                        hid2 = []
                        for mh in range(NH):
                            p2 = cps_mlp.tile([P, 512], F32, space="PSUM", tag="p2")
                            for k in range(NH):
                                nc.tensor.matmul(
                                    out=p2[:, :gw],
                                    lhsT=W[f"w2a{li}_{k}"][:, mh * P:(mh + 1) * P],
                                    rhs=pooledT[k][:, :gw],
                                    start=(k == 0), stop=(k == NH - 1))
                            h2 = csb.tile([P, 512], BF16, tag=f"h2_{mh}",
                                          name=f"h2_{mh}")
                            nc.scalar.activation(out=h2[:, :gw], in_=p2[:, :gw],
                                                 func=PRELU,
                                                 bias=W[f"b2a{li}"][:, mh:mh + 1],
                                                 alpha=ALPHA)
                            hid2.append(h2)
                        pno = cps_mlp.tile([P, 512], F32, space="PSUM", tag="p2")
                        for k in range(NH):
                            nc.tensor.matmul(out=pno[:dout, :gw],
                                             lhsT=W[f"w2b{li}_{k}"][:],
                                             rhs=hid2[k][:, :gw],
                                             start=(k == 0), stop=(k == NH - 1))
                        noT = csb.tile([dout, 512], BF16, tag="noT")
                        nc.scalar.activation(out=noT[:, :gw], in_=pno[:dout, :gw],
                                             func=PRELU, bias=W[f"b2b{li}"][:, :1],
                                             alpha=ALPHA)
                        if li < 3:
                            for q in range(tn):
                                ptr3 = cps_tr.tile([P, P], BF16, space="PSUM", tag="ptr2")
                                nc.tensor.transpose(out=ptr3[:, :dout],
                                                    in_=noT[:, q * P:(q + 1) * P],
                                                    identity=ident_bf[:])
                                ent2 = csb.tile([P, P], BF16, tag="ent2")
                                nc.vector.tensor_copy(out=ent2[:, :dout],
                                                      in_=ptr3[:, :dout])
                                r0 = (t0 + q) * P
                                nc.sync.dma_start(
                                    out=agins[li + 1][r0:r0 + P, :dout],
                                    in_=ent2[:, :dout])
                        else:
                            phd = cps_mlp.tile([4, 512], F32, space="PSUM", tag="phd")
                            nc.tensor.matmul(out=phd[:, :gw], lhsT=W["wbb"][:],
                                             rhs=noT[:, :gw], start=True, stop=True)
                            ho = csb.tile([4, 512], F32, tag="ho")
                            nc.scalar.activation(out=ho[:, :gw], in_=phd[:, :gw],
                                                 func=PRELU, bias=W["bbb"][:, :1],
                                                 alpha=ALPHA)
                            nc.sync.dma_start(out=out[:, t0 * P:t0 * P + gw],
                                              in_=ho[:, :gw])

                # ---- phase D: AllGather new object table ----
                if li < 3:
                    nc.gpsimd.collective_compute(
                        "AllGather", mybir.AluOpType.bypass, replica_groups=GRPS,
                        ins=[agins[li + 1][0:OSP, :]],
                        outs=[tabs[li + 1][P:P + OG, :]])

    nc.compile()
    return nc


# ---------------------------------------------------------------------------
# Entry point
# ---------------------------------------------------------------------------

_CACHE = {}


def kernel(**inputs) -> np.ndarray:
    cfg, in_maps = preprocess(inputs)
    key = tuple(sorted((k, v) for k, v in cfg.items()))
    if key not in _CACHE:
        _CACHE[key] = build_kernel(cfg)
    nc = _CACHE[key]
    res = run_bass_kernel_spmd(nc, in_maps, list(range(NC)))
    O, OS = cfg["O"], cfg["OS"]
    full = np.zeros((4, O), np.float32)
    for c in range(NC):
        full[:, c * OS:(c + 1) * OS] = res.results[c]["out"][:, :OS]
    return np.ascontiguousarray(full.T)


# revision 7
# speedup vs baseline: 10.3727x; 1.1532x over previous
"""Trainium2 Bass kernel for nn_NDNRefinement (4-layer GNN message passing).

Strategy (8 NeuronCores), V2:
- Shard triples by subject core (s // OS). Within a core, triples are sorted
  by (dest core of object, o_local) and padded so each destination section is
  exactly S_B entries. new_o rows are then written CONTIGUOUSLY into the
  AllToAll send buffer (no scatter); row indices all fit int16.
- All gathers use single batched dma_gather instructions:
  * phase A inputs: transpose-gather (feature-major) of subject rows from the
    local object table and object rows from the replicated global table (two
    overlapping windows handle >32k rows; out-of-window entries hit dedicated
    zero rows and the two partial gathers accumulate in the matmul).
  * pooling: entry-major 768-row gathers from stage/recv per object tile,
    pooled with one-hot matmuls (iota + is_equal).
- Index schedules are resident in SBUF (loaded once, shared by all layers).
- Per-object MLPs are data-parallel over the object shard; the new object
  table is AllGathered between layers into a window-padded table.
"""

import os
import numpy as np
import ml_dtypes

import concourse.bass as bass
import concourse.bacc as bacc
import concourse.tile as tile
from concourse import mybir
from concourse import library_config
from concourse.bass_utils import run_bass_kernel_spmd
from concourse.masks import make_identity

BF16 = mybir.dt.bfloat16
F32 = mybir.dt.float32
I32 = mybir.dt.int32
I16 = mybir.dt.int16
P = 128
NC = 8
ALPHA = 0.2
DF = 128                       # object-table feature width (padded)
IDXCAP = 32768                 # int16 index window size

# (din, h, dout) per layer
DIMS = [(64, 512, 128), (128, 512, 128), (128, 512, 128), (128, 128, 128)]


def _rup(x, m):
    return ((int(x) + m - 1) // m) * m


def _pack16(vals, total):
    """int16 index list -> [128, total//16] wrapped tile (j -> [j%16, j//16]),
    replicated to 128 partitions."""
    a = np.zeros((total,), np.int16)
    a[:len(vals)] = vals
    w = a.reshape(total // 16, 16).T          # [16, total//16]
    return np.tile(w, (8, 1)).copy()          # [128, total//16]


# ---------------------------------------------------------------------------
# Host preprocessing
# ---------------------------------------------------------------------------

def preprocess(inputs):
    obj_vecs = np.asarray(inputs["obj_vecs"], np.float32)
    pred_vecs = np.asarray(inputs["pred_vecs"], np.float32)
    pred_boxes = np.asarray(inputs["pred_boxes"], np.float32)
    s_idx = np.asarray(inputs["s_idx"], np.int32)
    o_idx = np.asarray(inputs["o_idx"], np.int32)

    O = obj_vecs.shape[0]
    assert O % NC == 0
    OS = O // NC
    OSP = _rup(OS, P)
    NT = OSP // P
    OG = NC * OSP
    TABR = OG + 2 * P            # front + tail zero rows
    WB0 = max(0, TABR - IDXCAP)  # window B start row
    ZBI = OG + P - WB0           # window-B index of a tail zero row

    core = s_idx // OS
    percore = []
    S_B = 0
    for c in range(NC):
        idxs = np.where(core == c)[0]
        d = o_idx[idxs] // OS
        o_loc = o_idx[idxs] % OS
        order = np.lexsort((o_loc, d))
        idxs = idxs[order]
        d = d[order]
        o_loc = o_loc[order]
        bc = np.bincount(d, minlength=NC)
        S_B = max(S_B, int(bc.max()))
        percore.append(dict(idxs=idxs, d=d, o_loc=o_loc, bc=bc,
                            s_loc=s_idx[idxs] - c * OS))
    NCSB = NC * S_B
    assert NCSB <= IDXCAP, f"NCSB={NCSB} exceeds int16 window"
    TP = _rup(NCSB, 512)
    NB = TP // 512

    # per-object counts
    cnt = np.bincount(s_idx, minlength=O) + np.bincount(o_idx, minlength=O)
    inv_cnt = (1.0 / np.maximum(cnt, 1)).astype(np.float32)

    # entry position arrays per core
    pos_of = []           # per core: position of each (sorted) real triple
    for c in range(NC):
        pc = percore[c]
        first = np.concatenate([[0], np.cumsum(pc["bc"])[:-1]])
        rank = np.arange(len(pc["d"])) - first[pc["d"]]
        pos = (pc["d"] * S_B + rank).astype(np.int64)
        pos_of.append(pos)

    # max pooling loads
    max_s_load = 1
    max_o_load = 1
    for c in range(NC):
        pc = percore[c]
        sload = np.bincount(pc["s_loc"] // P, minlength=NT)
        max_s_load = max(max_s_load, int(sload.max()))
    for c in range(NC):
        oload = np.zeros((NT,), np.int64)
        for sc in range(NC):
            pc = percore[sc]
            m = pc["d"] == c
            oload += np.bincount(pc["o_loc"][m] // P, minlength=NT)
        max_o_load = max(max_o_load, int(oload.max()))
    PS = -(-max_s_load // P)
    PO = -(-max_o_load // P)

    cfg = dict(O=O, OS=OS, OSP=OSP, NT=NT, OG=OG, TABR=TABR, WB0=WB0,
               S_B=S_B, NCSB=NCSB, TP=TP, NB=NB, PS=PS, PO=PO)

    # ---- weights, shared across cores ----
    bf = ml_dtypes.bfloat16
    shared = {}
    w_emb = np.zeros((68, DF), np.float32)
    w_emb[:, :64] = np.asarray(inputs["W_emb"], np.float32)
    shared["w_emb"] = w_emb.astype(bf)
    b_emb = np.zeros((DF, 1), np.float32)
    b_emb[:64, 0] = np.asarray(inputs["b_emb"], np.float32)
    shared["b_emb"] = b_emb
    for li, (din, h, dout) in enumerate(DIMS):
        b1b = np.asarray(inputs[f"b1b{li}"], np.float32)
        shared[f"w1a{li}"] = np.asarray(inputs[f"W1a{li}"], np.float32).astype(bf)
        shared[f"w1b{li}"] = np.asarray(inputs[f"W1b{li}"], np.float32).astype(bf)
        shared[f"w2a{li}"] = np.asarray(inputs[f"W2a{li}"], np.float32).astype(bf)
        shared[f"w2b{li}"] = np.asarray(inputs[f"W2b{li}"], np.float32).astype(bf)
        shared[f"b1a{li}"] = np.asarray(inputs[f"b1a{li}"], np.float32).reshape(-1, P).T.copy()
        shared[f"b1bp{li}"] = b1b[h:h + dout].reshape(-1, 1).copy()
        shared[f"b1bs{li}"] = np.broadcast_to(b1b[:h].astype(bf), (P, h)).copy()
        shared[f"b1bo{li}"] = np.broadcast_to(b1b[h + dout:].astype(bf), (P, h)).copy()
        shared[f"b2a{li}"] = np.asarray(inputs[f"b2a{li}"], np.float32).reshape(-1, P).T.copy()
        shared[f"b2b{li}"] = np.asarray(inputs[f"b2b{li}"], np.float32).reshape(-1, 1).copy()
    shared["wbb"] = np.asarray(inputs["W_bb"], np.float32).astype(bf)
    shared["bbb"] = np.asarray(inputs["b_bb"], np.float32).reshape(-1, 1)

    x_full = np.concatenate([obj_vecs, pred_boxes], axis=1)      # (O, 68)

    in_maps = []
    for c in range(NC):
        pc = percore[c]
        pos = pos_of[c]
        n = len(pos)
        m = {}
        xT = np.zeros((68, OSP), bf)
        xT[:, :OS] = x_full[c * OS:(c + 1) * OS].T.astype(bf)
        m["xt"] = xT

        pT = np.zeros((64, TP), bf)
        pT[:, pos] = pred_vecs[pc["idxs"]].T.astype(bf)
        m["pred0"] = pT

        # per-entry gather indices (by position)
        ent_s = np.full((TP,), OSP, np.int16)
        ent_s[pos] = pc["s_loc"].astype(np.int16)
        g = (pc["d"] * OSP + pc["o_loc"]).astype(np.int64)
        ent_oa = np.zeros((TP,), np.int16)
        ent_ob = np.full((TP,), ZBI, np.int16)
        loA = g <= IDXCAP - 1 - P
        ent_oa[pos[loA]] = (g[loA] + P).astype(np.int16)
        ent_oa[pos[~loA]] = 0
        ent_ob[pos[~loA]] = (g[~loA] + P - WB0).astype(np.int16)
        m["sidx"] = _pack16(ent_s, TP)
        m["oaidx"] = _pack16(ent_oa, TP)
        m["obidx"] = _pack16(ent_ob, TP)

        # s-pool schedule: per tile, positions of entries with s in tile
        sp16 = np.zeros((NT, PS * P), np.int16)
        spl = np.full((NT, PS * P), -1, np.int32)
        tsel = pc["s_loc"] // P
        for t in range(NT):
            mask = tsel == t
            k = int(mask.sum())
            sp16[t, :k] = pos[mask].astype(np.int16)
            spl[t, :k] = (pc["s_loc"][mask] - t * P).astype(np.int32)
        m["spool"] = np.concatenate(
            [_pack16(sp16[t], PS * P) for t in range(NT)], axis=1)
        m["sloc"] = spl.reshape(NT * PS, P).T.copy()

        # o-pool schedule: recv rows for my tiles
        rows_all, locs_all = [], []
        for sc in range(NC):
            qc = percore[sc]
            mm = qc["d"] == c
            rows_all.append(pos_of[sc][mm] - c * S_B + sc * S_B)
            locs_all.append(qc["o_loc"][mm])
        rows_all = np.concatenate(rows_all)
        locs_all = np.concatenate(locs_all)
        op16 = np.zeros((NT, PO * P), np.int16)
        opl = np.full((NT, PO * P), -1, np.int32)
        tsel = locs_all // P
        for t in range(NT):
            mask = tsel == t
            k = int(mask.sum())
            op16[t, :k] = rows_all[mask].astype(np.int16)
            opl[t, :k] = (locs_all[mask] - t * P).astype(np.int32)
        m["opool"] = np.concatenate(
            [_pack16(op16[t], PO * P) for t in range(NT)], axis=1)
        m["oloc"] = opl.reshape(NT * PO, P).T.copy()

        iv = np.zeros((OSP,), np.float32)
        iv[:OS] = inv_cnt[c * OS:(c + 1) * OS]
        m["invc"] = iv.reshape(NT, P).T.copy()
        m.update(shared)
        in_maps.append(m)

    return cfg, in_maps


# ---------------------------------------------------------------------------
# Kernel builder
# ---------------------------------------------------------------------------

def build_kernel(cfg):
    OSP, NT, OG, TABR, WB0 = (cfg["OSP"], cfg["NT"], cfg["OG"], cfg["TABR"],
                              cfg["WB0"])
    NCSB, TP, NB, PS, PO = (cfg["NCSB"], cfg["TP"], cfg["NB"], cfg["PS"],
                            cfg["PO"])
    WA = min(IDXCAP, TABR)       # window A size
    WBN = TABR - WB0             # window B size
    SW = OSP + P                 # local (subject) window size

    nc = bacc.Bacc("TRN2", target_bir_lowering=False, debug=False,
                   num_devices=NC, num_swdge_queues=4)

    # ---- parameters ----
    xt = nc.declare_dram_parameter("xt", [68, OSP], BF16, isOutput=False)
    pred0 = nc.declare_dram_parameter("pred0", [64, TP], BF16, isOutput=False)
    sidx = nc.declare_dram_parameter("sidx", [P, TP // 16], I16, isOutput=False)
    oaidx = nc.declare_dram_parameter("oaidx", [P, TP // 16], I16, isOutput=False)
    obidx = nc.declare_dram_parameter("obidx", [P, TP // 16], I16, isOutput=False)
    spool = nc.declare_dram_parameter("spool", [P, NT * PS * 8], I16, isOutput=False)
    opool = nc.declare_dram_parameter("opool", [P, NT * PO * 8], I16, isOutput=False)
    sloc = nc.declare_dram_parameter("sloc", [P, NT * PS], I32, isOutput=False)
    oloc = nc.declare_dram_parameter("oloc", [P, NT * PO], I32, isOutput=False)
    invc = nc.declare_dram_parameter("invc", [P, NT], F32, isOutput=False)

    w_emb = nc.declare_dram_parameter("w_emb", [68, DF], BF16, isOutput=False)
    b_emb = nc.declare_dram_parameter("b_emb", [DF, 1], F32, isOutput=False)
    wp = {}
    for li, (din, h, dout) in enumerate(DIMS):
        wp[f"w1a{li}"] = nc.declare_dram_parameter(f"w1a{li}", [3 * din, h], BF16, isOutput=False)
        wp[f"w1b{li}"] = nc.declare_dram_parameter(f"w1b{li}", [h, 2 * h + dout], BF16, isOutput=False)
        wp[f"w2a{li}"] = nc.declare_dram_parameter(f"w2a{li}", [h, h], BF16, isOutput=False)
        wp[f"w2b{li}"] = nc.declare_dram_parameter(f"w2b{li}", [h, dout], BF16, isOutput=False)
        wp[f"b1a{li}"] = nc.declare_dram_parameter(f"b1a{li}", [P, h // P], F32, isOutput=False)
        wp[f"b1bp{li}"] = nc.declare_dram_parameter(f"b1bp{li}", [dout, 1], F32, isOutput=False)
        wp[f"b1bs{li}"] = nc.declare_dram_parameter(f"b1bs{li}", [P, h], BF16, isOutput=False)
        wp[f"b1bo{li}"] = nc.declare_dram_parameter(f"b1bo{li}", [P, h], BF16, isOutput=False)
        wp[f"b2a{li}"] = nc.declare_dram_parameter(f"b2a{li}", [P, h // P], F32, isOutput=False)
        wp[f"b2b{li}"] = nc.declare_dram_parameter(f"b2b{li}", [dout, 1], F32, isOutput=False)
    wbb = nc.declare_dram_parameter("wbb", [P, 4], BF16, isOutput=False)
    bbb = nc.declare_dram_parameter("bbb", [4, 1], F32, isOutput=False)

    out = nc.declare_dram_parameter("out", [4, OSP], F32, isOutput=True)

    # ---- internal DRAM ----
    tabs, agins = [], []
    for li in range(4):
        tabs.append(nc.dram_tensor(f"tab{li}", [TABR, DF], BF16,
                                   addr_space="Shared"))
        agins.append(nc.dram_tensor(f"agin{li}", [OSP + P, DF], BF16))
    preds = [pred0]
    for li in range(1, 4):
        preds.append(nc.dram_tensor(f"pred{li}", [P, TP], BF16))
    stages, sends, recvs = [], [], []
    for li, (din, h, dout) in enumerate(DIMS):
        stages.append(nc.dram_tensor(f"stage{li}", [TP, h], BF16))
        sends.append(nc.dram_tensor(f"send{li}", [TP, h], BF16))
        recvs.append(nc.dram_tensor(f"recv{li}", [NCSB, h], BF16))

    PRELU = mybir.ActivationFunctionType.Prelu
    COPY = mybir.ActivationFunctionType.Copy
    GRPS = [list(range(NC))]

    with tile.TileContext(nc) as tc:
        with tc.tile_pool(name="cst", bufs=1) as cst:
            # constants (iota needs the standard gpsimd library: do it first)
            ident = cst.tile([P, P], F32)
            make_identity(nc, ident[:])
            ident_bf = cst.tile([P, P], BF16)
            nc.vector.tensor_copy(out=ident_bf[:], in_=ident[:])
            iota = cst.tile([P, P], I32)
            nc.gpsimd.iota(iota[:], pattern=[[1, P]], base=0,
                           channel_multiplier=0)
            zt = cst.tile([P, DF], BF16)
            nc.gpsimd.memset(zt[:], 0)
            nc.gpsimd.load_library(library_config.mlp)

            # zero rows of the windowed tables (once per call)
            for li in range(4):
                nc.sync.dma_start(out=tabs[li][0:P, :], in_=zt[:])
                nc.sync.dma_start(out=tabs[li][OG + P:OG + 2 * P, :], in_=zt[:])
                nc.sync.dma_start(out=agins[li][OSP:OSP + P, :], in_=zt[:])

            W = {}

            def load_w(name, src_ap, hh, ww, dt):
                t = cst.tile([hh, ww], dt, tag=name)
                nc.sync.dma_start(out=t[:], in_=src_ap)
                W[name] = t

            load_w("w_emb", w_emb[:, :], 68, DF, BF16)
            load_w("b_emb", b_emb[:, :], DF, 1, F32)
            load_w("wbb", wbb[:, :], P, 4, BF16)
            load_w("bbb", bbb[:, :], 4, 1, F32)
            load_w("invc", invc[:, :], P, NT, F32)
            load_w("sidx", sidx[:, :], P, TP // 16, I16)
            load_w("oaidx", oaidx[:, :], P, TP // 16, I16)
            load_w("obidx", obidx[:, :], P, TP // 16, I16)
            load_w("spool", spool[:, :], P, NT * PS * 8, I16)
            load_w("opool", opool[:, :], P, NT * PO * 8, I16)
            load_w("sloc", sloc[:, :], P, NT * PS, I32)
            load_w("oloc", oloc[:, :], P, NT * PO, I32)
            for li, (din, h, dout) in enumerate(DIMS):
                for ki in range(3):
                    load_w(f"w1a{li}_c{ki}",
                           wp[f"w1a{li}"][ki * din:(ki + 1) * din, :],
                           din, h, BF16)
                for k in range(h // P):
                    load_w(f"w1b{li}_{k}", wp[f"w1b{li}"][k * P:(k + 1) * P, :],
                           P, 2 * h + dout, BF16)
                    load_w(f"w2a{li}_{k}", wp[f"w2a{li}"][k * P:(k + 1) * P, :],
                           P, h, BF16)
                    load_w(f"w2b{li}_{k}", wp[f"w2b{li}"][k * P:(k + 1) * P, :],
                           P, dout, BF16)
                load_w(f"b1a{li}", wp[f"b1a{li}"][:, :], P, h // P, F32)
                load_w(f"b1bp{li}", wp[f"b1bp{li}"][:, :], dout, 1, F32)
                load_w(f"b1bs{li}", wp[f"b1bs{li}"][:, :], P, h, BF16)
                load_w(f"b1bo{li}", wp[f"b1bo{li}"][:, :], P, h, BF16)
                load_w(f"b2a{li}", wp[f"b2a{li}"][:, :], P, h // P, F32)
                load_w(f"b2b{li}", wp[f"b2b{li}"][:, :], dout, 1, F32)

            # ---------------- embedding phase ----------------
            NEB = -(-OSP // 512)
            with (
                tc.tile_pool(name="esb", bufs=3) as esb,
                tc.tile_pool(name="eps", bufs=3, space="PSUM") as eps,
            ):
                for b in range(NEB):
                    c0 = b * 512
                    w = min(512, OSP - c0)
                    xin = esb.tile([68, 512], BF16, tag="xin")
                    nc.sync.dma_start(out=xin[:, :w], in_=xt[:, c0:c0 + w])
                    pse = eps.tile([DF, 512], F32, space="PSUM", tag="pse")
                    nc.tensor.matmul(out=pse[:, :w], lhsT=W["w_emb"][:],
                                     rhs=xin[:, :w], start=True, stop=True)
                    ebt = esb.tile([DF, 512], BF16, tag="ebt")
                    nc.scalar.activation(out=ebt[:, :w], in_=pse[:, :w],
                                         func=PRELU, bias=W["b_emb"][:, :1],
                                         alpha=ALPHA)
                    for q in range(-(-w // P)):
                        qw = min(P, w - q * P)
                        ptr = eps.tile([P, DF], BF16, space="PSUM", tag="ptr")
                        nc.tensor.transpose(out=ptr[:qw, :],
                                            in_=ebt[:, q * P:q * P + qw],
                                            identity=ident_bf[:])
                        ent = esb.tile([P, DF], BF16, tag="ent")
                        nc.vector.tensor_copy(out=ent[:qw, :], in_=ptr[:qw, :])
                        nc.sync.dma_start(
                            out=agins[0][c0 + q * P:c0 + q * P + qw, :],
                            in_=ent[:qw, :])
            nc.gpsimd.collective_compute(
                "AllGather", mybir.AluOpType.bypass, replica_groups=GRPS,
                ins=[agins[0][0:OSP, :]], outs=[tabs[0][P:P + OG, :]])

            # ---------------- layers ----------------
            _MAXL = int(os.environ.get("KMAXL", "4"))
            for li, (din, h, dout) in enumerate(DIMS[:_MAXL]):
                tab_in, agin_in = tabs[li], agins[li]
                pred_in = preds[li]
                stage, send, recv = stages[li], sends[li], recvs[li]
                NH = h // P
                s_cols = (0, h)
                p_cols = (h, h + dout)
                o_cols = (h + dout, 2 * h + dout)

                # ---- phase A: triple MLP ----
                with (
                    tc.tile_pool(name=f"asb{li}", bufs=3) as asb,
                    tc.tile_pool(name=f"apshid{li}", bufs=NH, space="PSUM") as aps_hid,
                    tc.tile_pool(name=f"apsout{li}", bufs=2, space="PSUM") as aps_out,
                ):
                    for j in range(NB):
                        ic0 = j * 32
                        sT = asb.tile([P, 1, 512], BF16, tag="sT")
                        nc.gpsimd.dma_gather(
                            sT[:], agin_in[0:SW, :],
                            W["sidx"][:, ic0:ic0 + 32], 512, 512, DF,
                            transpose=True, queue_num=0)
                        oTA = asb.tile([P, 1, 512], BF16, tag="oTA")
                        nc.gpsimd.dma_gather(
                            oTA[:], tab_in[0:WA, :],
                            W["oaidx"][:, ic0:ic0 + 32], 512, 512, DF,
                            transpose=True, queue_num=1)
                        oTB = asb.tile([P, 1, 512], BF16, tag="oTB")
                        nc.gpsimd.dma_gather(
                            oTB[:], tab_in[WB0:WB0 + WBN, :],
                            W["obidx"][:, ic0:ic0 + 32], 512, 512, DF,
                            transpose=True, queue_num=2)
                        oT = asb.tile([P, 512], BF16, tag="oT")
                        nc.vector.tensor_tensor(
                            out=oT[:], in0=oTA[:, 0, :], in1=oTB[:, 0, :],
                            op=mybir.AluOpType.add)
                        pT = asb.tile([din, 512], BF16, tag="pT")
                        nc.sync.dma_start(out=pT[:],
                                          in_=pred_in[:din, 512 * j:512 * (j + 1)])

                        # hid
                        hidT = []
                        for mh in range(NH):
                            ph = aps_hid.tile([P, 512], F32, space="PSUM", tag="ph")
                            for ki, src in enumerate(
                                    (sT[:din, 0, :], pT[:], oT[:din, :])):
                                nc.tensor.matmul(
                                    out=ph[:],
                                    lhsT=W[f"w1a{li}_c{ki}"][:, mh * P:(mh + 1) * P],
                                    rhs=src,
                                    start=(ki == 0), stop=(ki == 2))
                            ht = asb.tile([P, 512], BF16, tag=f"hidT{mh}",
                                          name=f"hidT{mh}")
                            nc.scalar.activation(out=ht[:], in_=ph[:], func=PRELU,
                                                 bias=W[f"b1a{li}"][:, mh:mh + 1],
                                                 alpha=ALPHA)
                            hidT.append(ht)

                        # new_s / new_o (entry-major, contiguous rows)
                        for (cols, bname, dst) in ((s_cols, f"b1bs{li}", stage),
                                                   (o_cols, f"b1bo{li}", send)):
                            for e in range(4):
                                po = aps_out.tile([P, 512], F32, space="PSUM", tag="po")
                                for k in range(NH):
                                    nc.tensor.matmul(
                                        out=po[:, :h],
                                        lhsT=hidT[k][:, e * P:(e + 1) * P],
                                        rhs=W[f"w1b{li}_{k}"][:, cols[0]:cols[1]],
                                        start=(k == 0), stop=(k == NH - 1))
                                nc.vector.tensor_tensor(
                                    out=po[:, :h], in0=po[:, :h], in1=W[bname][:],
                                    op=mybir.AluOpType.add)
                                ov = asb.tile([P, 512], BF16, tag="ov")
                                nc.scalar.activation(out=ov[:, :h], in_=po[:, :h],
                                                     func=PRELU, alpha=ALPHA)
                                r0 = 512 * j + e * P
                                nc.sync.dma_start(out=dst[r0:r0 + P, :],
                                                  in_=ov[:, :h])

                        # new_p (feature-major), not needed after last layer
                        if li < 3:
                            pp = aps_out.tile([P, 512], F32, space="PSUM", tag="po")
                            for k in range(NH):
                                nc.tensor.matmul(
                                    out=pp[:dout, :],
                                    lhsT=W[f"w1b{li}_{k}"][:, p_cols[0]:p_cols[1]],
                                    rhs=hidT[k][:],
                                    start=(k == 0), stop=(k == NH - 1))
                            pv = asb.tile([dout, 512], BF16, tag="pv")
                            nc.scalar.activation(out=pv[:], in_=pp[:dout, :],
                                                 func=PRELU,
                                                 bias=W[f"b1bp{li}"][:, :1],
                                                 alpha=ALPHA)
                            nc.sync.dma_start(
                                out=preds[li + 1][:, 512 * j:512 * (j + 1)],
                                in_=pv[:])

                # ---- phase B: AllToAll ----
                nc.gpsimd.collective_compute(
                    "AllToAll", mybir.AluOpType.bypass, replica_groups=GRPS,
                    ins=[send[0:NCSB, :]], outs=[recv[:]])

                # ---- phase C: pooling + object MLP ----
                with (
                    tc.tile_pool(name=f"cgth{li}", bufs=2) as cgth,
                    tc.tile_pool(name=f"csb{li}", bufs=3) as csb,
                    tc.tile_pool(name=f"cpool{li}", bufs=2, space="PSUM") as cps_pool,
                    tc.tile_pool(name=f"ctr{li}", bufs=2, space="PSUM") as cps_tr,
                    tc.tile_pool(name=f"cmlp{li}", bufs=2, space="PSUM") as cps_mlp,
                ):
                    ng = -(-NT // 4)
                    for grp in range(ng):
                        t0 = grp * 4
                        tn = min(4, NT - t0)
                        gw = tn * P
                        pooledT = [csb.tile([P, 512], BF16, tag=f"pooledT{k}",
                                            name=f"pooledT{k}")
                                   for k in range(NH)]
                        for tt in range(t0, t0 + tn):
                            vs = cgth.tile([P, PS, h], BF16, tag="vs")
                            nc.gpsimd.dma_gather(
                                vs[:], stage[:, :],
                                W["spool"][:, tt * PS * 8:(tt + 1) * PS * 8],
                                PS * P, PS * P, h,
                                queue_num=(2 * tt) % 4)
                            vo = cgth.tile([P, PO, h], BF16, tag="vo")
                            nc.gpsimd.dma_gather(
                                vo[:], recv[:, :],
                                W["opool"][:, tt * PO * 8:(tt + 1) * PO * 8],
                                PO * P, PO * P, h,
                                queue_num=(2 * tt + 1) % 4)
                            pps = cps_pool.tile([P, h], F32, space="PSUM", tag="pps")
                            nmm = 0
                            for (vals, locs, PN) in ((vs, "sloc", PS),
                                                     (vo, "oloc", PO)):
                                for k in range(PN):
                                    col = tt * PN + k
                                    oh = csb.tile([P, P], BF16, tag="oh")
                                    nc.vector.tensor_tensor(
                                        out=oh[:],
                                        in0=W[locs][:, col:col + 1].to_broadcast([P, P]),
                                        in1=iota[:], op=mybir.AluOpType.is_equal)
                                    nc.tensor.matmul(out=pps[:], lhsT=oh[:],
                                                     rhs=vals[:, k, :],
                                                     start=(nmm == 0),
                                                     stop=(nmm == (PS + PO) - 1))
                                    nmm += 1
                            pob = csb.tile([P, h], BF16, tag="pob")
                            nc.scalar.activation(out=pob[:], in_=pps[:], func=COPY,
                                                 scale=W["invc"][:, tt:tt + 1])
                            for k in range(NH):
                                ptr2 = cps_tr.tile([P, P], BF16, space="PSUM", tag="ptr2")
                                nc.tensor.transpose(out=ptr2[:],
                                                    in_=pob[:, k * P:(k + 1) * P],
                                                    identity=ident_bf[:])
                                nc.vector.tensor_copy(
                                    out=pooledT[k][:, (tt - t0) * P:(tt - t0 + 1) * P],
                                    in_=ptr2[:])
                        # object MLP on gw objects
                        hid2 = []
                        for mh in range(NH):
                            p2 = cps_mlp.tile([P, 512], F32, space="PSUM", tag="p2")
                            for k in range(NH):
                                nc.tensor.matmul(
                                    out=p2[:, :gw],
                                    lhsT=W[f"w2a{li}_{k}"][:, mh * P:(mh + 1) * P],
                                    rhs=pooledT[k][:, :gw],
                                    start=(k == 0), stop=(k == NH - 1))
                            h2 = csb.tile([P, 512], BF16, tag=f"h2_{mh}",
                                          name=f"h2_{mh}")
                            nc.scalar.activation(out=h2[:, :gw], in_=p2[:, :gw],
                                                 func=PRELU,
                                                 bias=W[f"b2a{li}"][:, mh:mh + 1],
                                                 alpha=ALPHA)
                            hid2.append(h2)
                        pno = cps_mlp.tile([P, 512], F32, space="PSUM", tag="p2")
                        for k in range(NH):
                            nc.tensor.matmul(out=pno[:dout, :gw],
                                             lhsT=W[f"w2b{li}_{k}"][:],
                                             rhs=hid2[k][:, :gw],
                                             start=(k == 0), stop=(k == NH - 1))
                        noT = csb.tile([dout, 512], BF16, tag="noT")
                        nc.scalar.activation(out=noT[:, :gw], in_=pno[:dout, :gw],
                                             func=PRELU, bias=W[f"b2b{li}"][:, :1],
                                             alpha=ALPHA)
                        if li < 3:
                            for q in range(tn):
                                ptr3 = cps_tr.tile([P, P], BF16, space="PSUM", tag="ptr2")
                                nc.tensor.transpose(out=ptr3[:, :dout],
                                                    in_=noT[:, q * P:(q + 1) * P],
                                                    identity=ident_bf[:])
                                ent2 = csb.tile([P, P], BF16, tag="ent2")
                                nc.vector.tensor_copy(out=ent2[:, :dout],
                                                      in_=ptr3[:, :dout])
                                r0 = (t0 + q) * P
                                nc.sync.dma_start(
                                    out=agins[li + 1][r0:r0 + P, :dout],
                                    in_=ent2[:, :dout])
                        else:
                            phd = cps_mlp.tile([4, 512], F32, space="PSUM", tag="phd")
                            nc.tensor.matmul(out=phd[:, :gw], lhsT=W["wbb"][:],
                                             rhs=noT[:, :gw], start=True, stop=True)
                            ho = csb.tile([4, 512], F32, tag="ho")
                            nc.scalar.activation(out=ho[:, :gw], in_=phd[:, :gw],
                                                 func=PRELU, bias=W["bbb"][:, :1],
                                                 alpha=ALPHA)
                            nc.sync.dma_start(out=out[:, t0 * P:t0 * P + gw],
                                              in_=ho[:, :gw])

                # ---- phase D: AllGather new object table ----
                if li < 3:
                    nc.gpsimd.collective_compute(
                        "AllGather", mybir.AluOpType.bypass, replica_groups=GRPS,
                        ins=[agins[li + 1][0:OSP, :]],
                        outs=[tabs[li + 1][P:P + OG, :]])

    nc.compile()
    return nc


# ---------------------------------------------------------------------------
# Entry point
# ---------------------------------------------------------------------------

_CACHE = {}


def kernel(**inputs) -> np.ndarray:
    cfg, in_maps = preprocess(inputs)
    key = tuple(sorted((k, v) for k, v in cfg.items()))
    if key not in _CACHE:
        _CACHE[key] = build_kernel(cfg)
    nc = _CACHE[key]
    res = run_bass_kernel_spmd(nc, in_maps, list(range(NC)))
    O, OS = cfg["O"], cfg["OS"]
    full = np.zeros((4, O), np.float32)
    for c in range(NC):
        full[:, c * OS:(c + 1) * OS] = res.results[c]["out"][:, :OS]
    return np.ascontiguousarray(full.T)
